# revision 2
# baseline (speedup 1.0000x reference)
"""Trainium2 Bass kernel for HNet attention (B=4, S=2048, H=768, 12 heads, RoPE, causal).

Sharding: 8 cores = 4 batches x 2 head-groups (6 heads each).
Wq/Wk/Wv split column-wise (head axis), Wo row-wise; host sums the two
partial o_proj outputs per batch (the "all-reduce" done at gather time).

v2 changes vs baseline:
  - bf16 for Q/K (post-RoPE), V, attn weights, o_proj weights: all
    score/PV matmuls run 1 cyc/row even for <256-col tiles.
  - causal mask folded into PSUM via a bias matmul (eye.T @ wedge) before
    the diagonal score matmul; exp of masked entries ~ e^-25 ~ 0.
    Removes the per-diag-tile gpsimd mask multiply and its serialization.
  - cos/sin as [128, 16*64] position-packed tables + stride-0 broadcast
    APs over the head axis (2 DMAs instead of 32, 1MB instead of 6.3MB).
  - packed QKV weights (one [768, 1152] tensor, 6 DMAs instead of 18).
  - merged output rows: one [128, 768] DMA per row-tile (16 instead of 32).
  - engine rebalance: RoPE muls/transposed copies/normalize on DVE;
    qr adds, V copies, sum-row staging, output copies on GpSimd.
"""

import os
import sys

import numpy as np

sys.path.insert(0, "/opt/trn_rl_repo")

from contextlib import ExitStack

import concourse.bacc as bacc
import concourse.tile as tile
from concourse import mybir
from concourse.bass_utils import run_bass_kernel_spmd

S = 2048
HID = 768
NH = 6            # heads per core
D = 64
F = NH * D        # 384 per-core feature slice
P = 128
SC = S // P       # 16
FC = HID // P     # 6
MC = F // P       # 3
QW = 512          # q strip width
NQ = S // QW      # 4
N_CORES = 8
ROPE_THETA = 10000.0
MASKV = -200.0    # causal wedge bias; exp(0.125 * -200) ~ 1.4e-11

F32 = mybir.dt.float32
F32R = mybir.dt.float32r
BF16 = mybir.dt.bfloat16
AF = mybir.ActivationFunctionType


def _h3(ap):
    """[P, F] -> [P, NH, D] view."""
    return ap.rearrange("p (h d) -> p h d", h=NH)


def build_program():
    nc = bacc.Bacc("TRN2", target_bir_lowering=False, debug=False,
                   num_devices=N_CORES)

    xT_d = nc.dram_tensor("xT", [HID, S], BF16, kind="ExternalInput").ap()
    wT_d = nc.dram_tensor("wT", [HID, 3 * F], BF16, kind="ExternalInput").ap()
    woT_d = nc.dram_tensor("woT", [F, HID], BF16, kind="ExternalInput").ap()
    cosP_d = nc.dram_tensor("cosP", [P, SC * D], F32, kind="ExternalInput").ap()
    sinP_d = nc.dram_tensor("sinP", [P, SC * D], F32, kind="ExternalInput").ap()
    eye_d = nc.dram_tensor("eye", [P, P], BF16, kind="ExternalInput").ap()
    wedge_d = nc.dram_tensor("wedge", [P, P], BF16, kind="ExternalInput").ap()
    on_d = nc.dram_tensor("ones6", [P, NH], BF16, kind="ExternalInput").ap()
    sel_d = nc.dram_tensor("sel33", [33, P], F32R, kind="ExternalInput").ap()
    out_d = nc.dram_tensor("out", [S, HID], F32, kind="ExternalOutput").ap()

    with tile.TileContext(nc) as tc, ExitStack() as ctx:
        const_pool = ctx.enter_context(tc.tile_pool(name="const", bufs=1))
        eye_sb = const_pool.tile([P, P], BF16, tag="eye")
        wedge_sb = const_pool.tile([P, P], BF16, tag="wedge")
        on_sb = const_pool.tile([P, NH], BF16, tag="ones6")
        sel_sb = const_pool.tile([33, P], F32R, tag="sel33")
        cosP_sb = const_pool.tile([P, SC * D], F32, tag="cosP")
        sinP_sb = const_pool.tile([P, SC * D], F32, tag="sinP")
        cos3 = cosP_sb.rearrange("p (s d) -> p s d", s=SC)
        sin3 = sinP_sb.rearrange("p (s d) -> p s d", s=SC)

        def load_consts():
            nc.sync.dma_start(cosP_sb[:], cosP_d[:])
            nc.sync.dma_start(sinP_sb[:], sinP_d[:])
            nc.sync.dma_start(eye_sb[:], eye_d[:])
            nc.sync.dma_start(wedge_sb[:], wedge_d[:])
            nc.sync.dma_start(on_sb[:], on_d[:])
            nc.sync.dma_start(sel_sb[:], sel_d[:])

        # persistent per-phase tensors
        qkT_pool = ctx.enter_context(tc.tile_pool(name="qkT", bufs=1))
        kTall = qkT_pool.tile([P, MC * S], BF16, tag="kTall", name="kTall")
        kT = [kTall[:, m * S:(m + 1) * S] for m in range(MC)]
        v_pool = ctx.enter_context(tc.tile_pool(name="vp", bufs=1))
        v_sb = [v_pool.tile([P, NH * 65], BF16, tag=f"v{s}", name=f"v{s}") for s in range(SC)]
        ao_pool = ctx.enter_context(tc.tile_pool(name="ao", bufs=2))
        woT_pool = ctx.enter_context(tc.tile_pool(name="woT", bufs=1))
        woT = [woT_pool.tile([P, HID], BF16, tag=f"woT{m}", name=f"woT{m}") for m in range(MC)]
        stg_pool2 = ctx.enter_context(tc.tile_pool(name="stgp", bufs=1))
        stg_t = stg_pool2.tile([33, MC * QW], F32R, tag="stg33", name="stg33")
        nc.vector.memset(stg_t[0:33, :].bitcast(F32), 0.0)
        for m in range(MC):
            nc.sync.dma_start(woT[m][:], woT_d[m * P:(m + 1) * P, :])

        # ---- single interleaved phase: proj-group(qc) then attention(qc) ----
        # PSUM banks (8): ps_qk 3 (pq/pk/pv) + ring 3 (sp/pt/bp/fin) + ps_pv 2
        with tc.tile_pool(name="xT", bufs=1) as xT_pool, \
             tc.tile_pool(name="wT", bufs=1) as wT_pool, \
             tc.tile_pool(name="rope", bufs=4) as rope_pool, \
             tc.tile_pool(name="ex", bufs=12) as ex_pool, \
             tc.tile_pool(name="stg", bufs=2) as stg_pool, \
             tc.tile_pool(name="sums", bufs=2) as sums_pool, \
             tc.tile_pool(name="ob", bufs=3) as ob_pool, \
             tc.tile_pool(name="ps_qk", bufs=1, space="PSUM") as ps_qk, \
             tc.tile_pool(name="ring", bufs=3, space="PSUM") as ring, \
             tc.tile_pool(name="ps_pv", bufs=2, space="PSUM") as ps_pv:

            xT = [xT_pool.tile([P, S], BF16, tag=f"xT{f}", name=f"xT{f}") for f in range(FC)]
            wT = [wT_pool.tile([P, 3 * F], BF16, tag=f"wT{f}", name=f"wTs{f}") for f in range(FC)]
            cs0 = slice(0, QW)
            for f in range(FC):
                nc.sync.dma_start(wT[f][:], wT_d[f * P:(f + 1) * P, :])
                nc.sync.dma_start(xT[f][:, cs0], xT_d[f * P:(f + 1) * P, cs0])
            load_consts()
            for c in range(1, NQ):
                cs = slice(c * QW, (c + 1) * QW)
                for f in range(FC):
                    nc.sync.dma_start(xT[f][:, cs], xT_d[f * P:(f + 1) * P, cs])
            for m in range(MC):
                nc.sync.dma_start(woT[m][:], woT_d[m * P:(m + 1) * P, :])

            def emit_proj(s):
                sl = slice(s * P, (s + 1) * P)
                pq = ps_qk.tile([P, F], F32, tag="pqkv", name="pq", bufs=3)
                pk = ps_qk.tile([P, F], F32, tag="pqkv", name="pk", bufs=3)
                pv_ = ps_qk.tile([P, F], F32, tag="pqkv", name="pv", bufs=3)
                for f in range(FC):
                    st, sp = (f == 0), (f == FC - 1)
                    lhs = xT[f][:, sl]
                    nc.tensor.matmul(pq[:], lhs, wT[f][:, 0:F], start=st, stop=sp)
                    nc.tensor.matmul(pk[:], lhs, wT[f][:, F:2 * F], start=st, stop=sp)
                    nc.tensor.matmul(pv_[:], lhs, wT[f][:, 2 * F:3 * F], start=st, stop=sp)
                return s, sl, pq, pk, pv_, None

            def emit_tail(state):
                s, sl, pq, pk, pv_, qTs = state
                qcol = (s % 4) * P
                cs_b = cos3[:, s, :].unsqueeze(1).broadcast_to([P, NH, D])
                sn_b = sin3[:, s, :].unsqueeze(1).broadcast_to([P, NH, D])
                dsts = {id(pq): (qTs, None), id(pk): (kT, sl)}
                # RoPE (natural layout): qr = q*cos + rot_half(q)*sin_signed
                for pp in (pq, pk):
                    dstT, dcol = dsts[id(pp)]
                    t1 = rope_pool.tile([P, F], F32, tag="t1", name="t1")
                    nc.vector.tensor_mul(_h3(t1)[:, :, :], _h3(pp)[:, :, :], cs_b)
                    t2 = rope_pool.tile([P, F], F32, tag="t2", name="t2")
                    nc.vector.tensor_mul(_h3(t2)[:, :, 0:32],
                                         _h3(pp)[:, :, 32:64],
                                         sn_b[:, :, 0:32])
                    nc.vector.tensor_mul(_h3(t2)[:, :, 32:64],
                                         _h3(pp)[:, :, 0:32],
                                         sn_b[:, :, 32:64])
                    qr = rope_pool.tile([P, F], BF16, tag="qr", name="qr")
                    nc.gpsimd.tensor_add(qr[:], t1[:], t2[:])
                    ptf = ps_qk.tile([P, 192], F32, tag="pqkv", name="pt", bufs=3)
                    pt = ptf.bitcast(BF16)
                    for m in range(MC):
                        nc.tensor.transpose(pt[:, m * P:(m + 1) * P],
                                            qr[:, m * P:(m + 1) * P],
                                            eye_sb[:])
                    # one 3D-AP copy into the combined kT/qT tile: the three
                    # m-tiles are (S or QW)-strided columns of one tensor
                    if pp is pk:
                        dst = kTall.rearrange("p (m s) -> p m s", m=MC)[:, :, dcol]
                    else:
                        dst = qTs[0].tensor.ap().rearrange(
                            "p (m s) -> p m s", m=MC)[:, :, qcol:qcol + P]
                    src = pt[:, 0:MC * P].rearrange("p (m s) -> p m s", m=MC)
                    nc.vector.tensor_copy(dst, src)
                # V with ones column per head: [V_h | 1] -> [P, NH*65] bf16
                v3 = v_sb[s].rearrange("p (h e) -> p h e", h=NH)
                nc.scalar.copy(v3[:, :, 0:64], _h3(pv_[:]))
                nc.gpsimd.tensor_copy(v3[:, :, 64:65],
                                      on_sb.rearrange("p (h o) -> p h o", h=NH))

            # ---- attention for strip qc, head pair m ----
            def attn_pair(qc, m, qTs, aoT, sums, stg):
                q0 = qc * QW
                last = 4 * qc + 3
                pvps = [ps_pv.tile([65, QW], F32, tag="pvp", name="pvp")
                        for _ in range(2)]
                def emit_pv(par, kc, qlo, n, ex):
                    h = 2 * m + par
                    nc.tensor.matmul(pvps[par][:, qlo - q0:QW],
                                     v_sb[kc][:, h * 65:h * 65 + 65],
                                     ex[:, 0:n],
                                     start=(kc == 0), stop=(kc == last))

                pend_pv = []
                for kc in range(last + 1):
                    k0 = kc * P
                    qlo = max(q0, k0)
                    n = q0 + QW - qlo
                    diag = k0 >= q0
                    exs = []
                    for par in range(2):
                        off = 64 * par
                        sp = ring.tile([P, QW], F32, tag="ring", name="sp")
                        kTb = kT[m][off:off + 64, k0:k0 + P]
                        qv = qTs[m][off:off + 64, :]
                        if diag:
                            # wedge bias then split score matmuls
                            nc.tensor.matmul(sp[:, 0:P], eye_sb[:],
                                             wedge_sb[:], start=True, stop=False)
                            nc.tensor.matmul(sp[:, 0:P], kTb,
                                             qv[:, qlo - q0:qlo - q0 + P],
                                             start=False, stop=True)
                            if n > P:
                                nc.tensor.matmul(sp[:, P:n], kTb,
                                                 qv[:, qlo - q0 + P:qlo - q0 + n],
                                                 start=True, stop=True)
                        else:
                            nc.tensor.matmul(sp[:, 0:n], kTb,
                                             qv[:, qlo - q0:qlo - q0 + n],
                                             start=True, stop=True)
                        ex = ex_pool.tile([P, QW], BF16, tag="ex", name="ex")
                        nc.scalar.activation(ex[:, 0:n], sp[:, 0:n],
                                             AF.Exp, scale=0.125)
                        exs.append((par, kc, qlo, n, ex))
                    # issue PV for the previous kc between this kc's scores
                    # and the next, so PE alternates scores/PV while the
                    # Activation engine streams exps one kc behind.
                    for args in pend_pv:
                        emit_pv(*args)
                    pend_pv = exs
                for args in pend_pv:
                    emit_pv(*args)
                mblk = slice(m * QW, (m + 1) * QW)
                for par in range(2):
                    with nc.allow_low_precision(reason="softmax sums"):
                        nc.vector.reciprocal(stg_t[32 * par:32 * par + 1, mblk],
                                             pvps[par][64:65, :])
                for par in range(2):
                    off = 64 * par
                    nc.vector.tensor_copy(aoT[m][off:off + 64, :],
                                          pvps[par][0:64, :])
                # bp[j, q] = inv(head for row j): sel33 selects row 0 (j<64)
                # or row 32 (j>=64); rows 1..31 of stg_t are always zero
                bp = ring.tile([P, QW], F32, tag="ring", name="bp")
                nc.tensor.matmul(bp[:, :], sel_sb[:], stg_t[0:33, mblk],
                                 start=True, stop=True)
                nc.vector.tensor_mul(aoT[m][:, :], aoT[m][:, :], bp[:, :])

            def attn_post(qc, aoT, sums):
                q0 = qc * QW
                for t in range(QW // P):
                    s0 = q0 + t * P
                    ob = ob_pool.tile([P, HID], F32, tag="ob", name="ob")
                    for half in range(2):
                        c0, c1 = half * F, half * F + F
                        fin = ring.tile([P, QW], F32, tag="ring", name="fin")
                        for m in range(MC):
                            nc.tensor.matmul(fin[:, 0:F],
                                             aoT[m][:, s0 - q0:s0 - q0 + P],
                                             woT[m][:, c0:c1],
                                             start=(m == 0), stop=(m == MC - 1))
                        nc.vector.tensor_copy(ob[:, c0:c1], fin[:, 0:F])
                    nc.sync.dma_start(out_d[s0:s0 + P, :], ob[:])

            def alloc_strip(qc):
                qTall = rope_pool.tile([P, MC * QW], BF16, tag="qTall", name="qTall")
                qTs = [qTall[:, m * QW:(m + 1) * QW] for m in range(MC)]
                aoT = [ao_pool.tile([P, QW], BF16, tag=f"aoTs{m}", name=f"aoTs{m}")
                       for m in range(MC)]
                return qTs, aoT, None, None

            # ---- emission: group 0, then zip(attention qc, proj group qc+1) ----
            strips = {0: alloc_strip(0)}
            pending = None
            pending_post = None
            for s in range(4):
                state = emit_proj(s)[:-1] + (strips[0][0],)
                if pending is not None:
                    emit_tail(pending)
                pending = state

            for qc in range(NQ):
                qTs, aoT, sums, stg = strips[qc]
                if qc + 1 < NQ:
                    strips[qc + 1] = alloc_strip(qc + 1)
                    nxt = list(range(4 * qc + 4, 4 * qc + 8))
                else:
                    nxt = []
                if not nxt and pending is not None:
                    emit_tail(pending)
                    pending = None
                for mp in range(MC + 1):
                    for s_i in nxt[mp:mp + 1]:
                        state = emit_proj(s_i)[:-1] + (strips[qc + 1][0],)
                        if pending is not None:
                            emit_tail(pending)
                        pending = state
                    if mp == 1 and pending_post is not None:
                        attn_post(*pending_post)
                        pending_post = None
                    if mp < MC:
                        attn_pair(qc, mp, qTs, aoT, sums, stg)
                pending_post = (qc, aoT, sums)
            attn_post(*pending_post)
    nc.compile()
    return nc


def _rope_tables():
    inv_freq = 1.0 / (ROPE_THETA ** (np.arange(0, D, 2, dtype=np.float32) / D))
    t = np.arange(S, dtype=np.float32)
    freqs = np.outer(t, inv_freq)                       # [S, 32]
    emb = np.concatenate([freqs, freqs], axis=-1)       # [S, 64]
    cos = np.cos(emb).astype(np.float32)
    sin = np.sin(emb).astype(np.float32)
    sin_signed = sin.copy()
    sin_signed[:, 0:32] *= -1.0                         # fold rotate_half sign
    # pack: tbl[p, s, d] = x[s*128 + p, d]  -> [128, 16*64]
    cosP = np.ascontiguousarray(
        cos.reshape(SC, P, D).transpose(1, 0, 2).reshape(P, SC * D))
    sinP = np.ascontiguousarray(
        sin_signed.reshape(SC, P, D).transpose(1, 0, 2).reshape(P, SC * D))
    return cosP, sinP


_STATE = {}


def _get_program():
    if "nc" not in _STATE:
        _STATE["nc"] = build_program()
    return _STATE["nc"]


def _bf16(x):
    import ml_dtypes
    return np.asarray(x, dtype=np.float32).astype(ml_dtypes.bfloat16)


def _make_in_maps(hidden_states, Wq, Wk, Wv, Wo):
    hs = np.asarray(hidden_states, dtype=np.float32)
    Wq = np.asarray(Wq, dtype=np.float32)
    Wk = np.asarray(Wk, dtype=np.float32)
    Wv = np.asarray(Wv, dtype=np.float32)
    Wo = np.asarray(Wo, dtype=np.float32)

    cosP, sinP = _rope_tables()
    eye = _bf16(np.eye(P, dtype=np.float32))
    wedge = _bf16(np.tril(np.full((P, P), MASKV, dtype=np.float32), k=-1))
    ones6 = _bf16(np.ones((P, NH), dtype=np.float32))
    sel33 = np.zeros((33, P), dtype=np.float32)
    sel33[0, 0:64] = 1.0
    sel33[32, 64:128] = 1.0

    in_maps = []
    for c in range(N_CORES):
        b, g = c // 2, c % 2
        cols = slice(g * F, (g + 1) * F)
        wT = np.concatenate([Wq[cols, :].T, Wk[cols, :].T, Wv[cols, :].T],
                            axis=1)                            # [768, 1152]
        in_maps.append({
            "xT": _bf16(np.ascontiguousarray(hs[b].T)),        # [768, S]
            "wT": _bf16(np.ascontiguousarray(wT)),
            "woT": _bf16(np.ascontiguousarray(Wo[:, cols].T)),  # [384, 768]
            "cosP": cosP,
            "sinP": sinP,
            "eye": eye,
            "wedge": wedge,
            "ones6": ones6,
            "sel33": sel33,
        })
    return in_maps


def run(hidden_states, Wq, Wk, Wv, Wo, trace=False, **trace_kw):
    nc = _get_program()
    in_maps = _make_in_maps(hidden_states, Wq, Wk, Wv, Wo)
    res = run_bass_kernel_spmd(nc, in_maps, core_ids=list(range(N_CORES)),
                               trace=trace, **trace_kw)
    B = 4
    out = np.empty((B, S, HID), dtype=np.float32)
    for b in range(B):
        out[b] = res.results[2 * b]["out"] + res.results[2 * b + 1]["out"]
    return out, res


def kernel(hidden_states, Wq, Wk, Wv, Wo):
    out, _ = run(hidden_states, Wq, Wk, Wv, Wo,
                 trace=bool(int(os.environ.get("KERNEL_TRACE", "0"))))
    return out


# revision 4
# speedup vs baseline: 1.0004x; 1.0004x over previous
"""Trainium2 Bass kernel for HNet attention (B=4, S=2048, H=768, 12 heads, RoPE, causal).

Sharding: 8 cores = 4 batches x 2 head-groups (6 heads each).
Wq/Wk/Wv split column-wise (head axis), Wo row-wise; host sums the two
partial o_proj outputs per batch (the "all-reduce" done at gather time).

v2 changes vs baseline:
  - bf16 for Q/K (post-RoPE), V, attn weights, o_proj weights: all
    score/PV matmuls run 1 cyc/row even for <256-col tiles.
  - causal mask folded into PSUM via a bias matmul (eye.T @ wedge) before
    the diagonal score matmul; exp of masked entries ~ e^-25 ~ 0.
    Removes the per-diag-tile gpsimd mask multiply and its serialization.
  - cos/sin as [128, 16*64] position-packed tables + stride-0 broadcast
    APs over the head axis (2 DMAs instead of 32, 1MB instead of 6.3MB).
  - packed QKV weights (one [768, 1152] tensor, 6 DMAs instead of 18).
  - merged output rows: one [128, 768] DMA per row-tile (16 instead of 32).
  - engine rebalance: RoPE muls/transposed copies/normalize on DVE;
    qr adds, V copies, sum-row staging, output copies on GpSimd.
"""

import os
import sys

import numpy as np

sys.path.insert(0, "/opt/trn_rl_repo")

from contextlib import ExitStack

import concourse.bacc as bacc
import concourse.tile as tile
from concourse import mybir
from concourse.bass_utils import run_bass_kernel_spmd

S = 2048
HID = 768
NH = 6            # heads per core
D = 64
F = NH * D        # 384 per-core feature slice
P = 128
SC = S // P       # 16
FC = HID // P     # 6
MC = F // P       # 3
QW = 512          # q strip width
NQ = S // QW      # 4
N_CORES = 8
ROPE_THETA = 10000.0
MASKV = -200.0    # causal wedge bias; exp(0.125 * -200) ~ 1.4e-11

F32 = mybir.dt.float32
F32R = mybir.dt.float32r
BF16 = mybir.dt.bfloat16
AF = mybir.ActivationFunctionType


def _h3(ap):
    """[P, F] -> [P, NH, D] view."""
    return ap.rearrange("p (h d) -> p h d", h=NH)


def build_program():
    nc = bacc.Bacc("TRN2", target_bir_lowering=False, debug=False,
                   num_devices=N_CORES)

    xT_d = nc.dram_tensor("xT", [HID, S], BF16, kind="ExternalInput").ap()
    wT_d = nc.dram_tensor("wT", [HID, 3 * F], BF16, kind="ExternalInput").ap()
    woT_d = nc.dram_tensor("woT", [F, HID], BF16, kind="ExternalInput").ap()
    cosP_d = nc.dram_tensor("cosP", [P, SC * D], F32, kind="ExternalInput").ap()
    sinP_d = nc.dram_tensor("sinP", [P, SC * D], F32, kind="ExternalInput").ap()
    eye_d = nc.dram_tensor("eye", [P, P], BF16, kind="ExternalInput").ap()
    wedge_d = nc.dram_tensor("wedge", [P, P], BF16, kind="ExternalInput").ap()
    on_d = nc.dram_tensor("ones6", [P, NH], BF16, kind="ExternalInput").ap()
    sel_d = nc.dram_tensor("sel33", [33, P], F32R, kind="ExternalInput").ap()
    out_d = nc.dram_tensor("out", [S, HID], F32, kind="ExternalOutput").ap()

    with tile.TileContext(nc) as tc, ExitStack() as ctx:
        const_pool = ctx.enter_context(tc.tile_pool(name="const", bufs=1))
        eye_sb = const_pool.tile([P, P], BF16, tag="eye")
        wedge_sb = const_pool.tile([P, P], BF16, tag="wedge")
        on_sb = const_pool.tile([P, NH], BF16, tag="ones6")
        sel_sb = const_pool.tile([33, P], F32R, tag="sel33")
        cosP_sb = const_pool.tile([P, SC * D], F32, tag="cosP")
        sinP_sb = const_pool.tile([P, SC * D], F32, tag="sinP")
        cos3 = cosP_sb.rearrange("p (s d) -> p s d", s=SC)
        sin3 = sinP_sb.rearrange("p (s d) -> p s d", s=SC)

        def load_consts():
            nc.sync.dma_start(cosP_sb[:], cosP_d[:])
            nc.sync.dma_start(sinP_sb[:], sinP_d[:])
            nc.sync.dma_start(eye_sb[:], eye_d[:])
            nc.sync.dma_start(wedge_sb[:], wedge_d[:])
            nc.sync.dma_start(on_sb[:], on_d[:])
            nc.sync.dma_start(sel_sb[:], sel_d[:])

        # persistent per-phase tensors
        qkT_pool = ctx.enter_context(tc.tile_pool(name="qkT", bufs=1))
        kTall = qkT_pool.tile([P, MC * S], BF16, tag="kTall", name="kTall")
        kT = [kTall[:, m * S:(m + 1) * S] for m in range(MC)]
        v_pool = ctx.enter_context(tc.tile_pool(name="vp", bufs=1))
        v_sb = [v_pool.tile([P, NH * 65], BF16, tag=f"v{s}", name=f"v{s}") for s in range(SC)]
        ao_pool = ctx.enter_context(tc.tile_pool(name="ao", bufs=2))
        woT_pool = ctx.enter_context(tc.tile_pool(name="woT", bufs=1))
        woT = [woT_pool.tile([P, HID], BF16, tag=f"woT{m}", name=f"woT{m}") for m in range(MC)]
        stg_pool2 = ctx.enter_context(tc.tile_pool(name="stgp", bufs=1))
        stg_t = stg_pool2.tile([33, MC * QW], F32R, tag="stg33", name="stg33")

        # ---- single interleaved phase: proj-group(qc) then attention(qc) ----
        # PSUM banks (8): ps_qk 3 (pq/pk/pv) + ring 3 (sp/pt/bp/fin) + ps_pv 2
        with tc.tile_pool(name="xT", bufs=1) as xT_pool, \
             tc.tile_pool(name="wT", bufs=1) as wT_pool, \
             tc.tile_pool(name="rope", bufs=6) as rope_pool, \
             tc.tile_pool(name="ex", bufs=12) as ex_pool, \
             tc.tile_pool(name="stg", bufs=2) as stg_pool, \
             tc.tile_pool(name="sums", bufs=2) as sums_pool, \
             tc.tile_pool(name="ob", bufs=3) as ob_pool, \
             tc.tile_pool(name="bps", bufs=2) as bps_pool, \
             tc.tile_pool(name="ps_qk", bufs=1, space="PSUM") as ps_qk, \
             tc.tile_pool(name="ring", bufs=3, space="PSUM") as ring, \
             tc.tile_pool(name="ps_pv", bufs=2, space="PSUM") as ps_pv:

            xT = [xT_pool.tile([P, S], BF16, tag=f"xT{f}", name=f"xT{f}") for f in range(FC)]
            wT = [wT_pool.tile([P, 3 * F], BF16, tag=f"wT{f}", name=f"wTs{f}") for f in range(FC)]
            cs0 = slice(0, QW)
            for f in range(FC):
                nc.sync.dma_start(wT[f][:], wT_d[f * P:(f + 1) * P, :])
                nc.sync.dma_start(xT[f][:, cs0], xT_d[f * P:(f + 1) * P, cs0])
            load_consts()
            for c in range(1, NQ):
                cs = slice(c * QW, (c + 1) * QW)
                for f in range(FC):
                    nc.sync.dma_start(xT[f][:, cs], xT_d[f * P:(f + 1) * P, cs])
            for m in range(MC):
                nc.sync.dma_start(woT[m][:], woT_d[m * P:(m + 1) * P, :])
            nc.vector.memset(stg_t[0:33, :].bitcast(F32), 0.0)

            def emit_proj(s):
                sl = slice(s * P, (s + 1) * P)
                pq = ps_qk.tile([P, F], F32, tag="pqkv", name="pq", bufs=3)
                pk = ps_qk.tile([P, F], F32, tag="pqkv", name="pk", bufs=3)
                pv_ = ps_qk.tile([P, F], F32, tag="pqkv", name="pv", bufs=3)
                for f in range(FC):
                    st, sp = (f == 0), (f == FC - 1)
                    lhs = xT[f][:, sl]
                    nc.tensor.matmul(pq[:], lhs, wT[f][:, 0:F], start=st, stop=sp)
                    nc.tensor.matmul(pk[:], lhs, wT[f][:, F:2 * F], start=st, stop=sp)
                    nc.tensor.matmul(pv_[:], lhs, wT[f][:, 2 * F:3 * F], start=st, stop=sp)
                return s, sl, pq, pk, pv_, None

            def emit_tail(state):
                s, sl, pq, pk, pv_, qTs = state
                qcol = (s % 4) * P
                cs_b = cos3[:, s, :].unsqueeze(1).broadcast_to([P, NH, D])
                sn_b = sin3[:, s, :].unsqueeze(1).broadcast_to([P, NH, D])
                dsts = {id(pq): (qTs, None), id(pk): (kT, sl)}
                # RoPE (natural layout): qr = q*cos + rot_half(q)*sin_signed
                for pp in (pq, pk):
                    dstT, dcol = dsts[id(pp)]
                    t1 = rope_pool.tile([P, F], F32, tag="t1", name="t1")
                    nc.vector.tensor_mul(_h3(t1)[:, :, :], _h3(pp)[:, :, :], cs_b)
                    t2 = rope_pool.tile([P, F], F32, tag="t2", name="t2")
                    nc.vector.tensor_mul(_h3(t2)[:, :, 0:32],
                                         _h3(pp)[:, :, 32:64],
                                         sn_b[:, :, 0:32])
                    nc.vector.tensor_mul(_h3(t2)[:, :, 32:64],
                                         _h3(pp)[:, :, 0:32],
                                         sn_b[:, :, 32:64])
                    qr = rope_pool.tile([P, F], BF16, tag="qr", name="qr")
                    nc.gpsimd.tensor_add(qr[:], t1[:], t2[:])
                    ptf = ps_qk.tile([P, 192], F32, tag="pqkv", name="pt", bufs=3)
                    pt = ptf.bitcast(BF16)
                    for m in range(MC):
                        nc.tensor.transpose(pt[:, m * P:(m + 1) * P],
                                            qr[:, m * P:(m + 1) * P],
                                            eye_sb[:])
                    # one 3D-AP copy into the combined kT/qT tile: the three
                    # m-tiles are (S or QW)-strided columns of one tensor
                    if pp is pk:
                        dst = kTall.rearrange("p (m s) -> p m s", m=MC)[:, :, dcol]
                    else:
                        dst = qTs[0].tensor.ap().rearrange(
                            "p (m s) -> p m s", m=MC)[:, :, qcol:qcol + P]
                    src = pt[:, 0:MC * P].rearrange("p (m s) -> p m s", m=MC)
                    nc.vector.tensor_copy(dst, src)
                # V with ones column per head: [V_h | 1] -> [P, NH*65] bf16
                v3 = v_sb[s].rearrange("p (h e) -> p h e", h=NH)
                nc.scalar.copy(v3[:, :, 0:64], _h3(pv_[:]))
                nc.gpsimd.tensor_copy(v3[:, :, 64:65],
                                      on_sb.rearrange("p (h o) -> p h o", h=NH))

            # ---- attention for strip qc, head pair m ----
            def attn_pair(qc, m, qTs, aoT, sums, stg):
                q0 = qc * QW
                last = 4 * qc + 3
                pvps = [ps_pv.tile([65, QW], F32, tag="pvp", name="pvp")
                        for _ in range(2)]
                def emit_pv(par, kc, qlo, n, ex):
                    h = 2 * m + par
                    nc.tensor.matmul(pvps[par][:, qlo - q0:QW],
                                     v_sb[kc][:, h * 65:h * 65 + 65],
                                     ex[:, 0:n],
                                     start=(kc == 0), stop=(kc == last))

                pend_pv = []
                for kc in range(last + 1):
                    k0 = kc * P
                    qlo = max(q0, k0)
                    n = q0 + QW - qlo
                    diag = k0 >= q0
                    exs = []
                    for par in range(2):
                        off = 64 * par
                        sp = ring.tile([P, QW], F32, tag="ring", name="sp")
                        kTb = kT[m][off:off + 64, k0:k0 + P]
                        qv = qTs[m][off:off + 64, :]
                        if diag:
                            # wedge bias then split score matmuls
                            nc.tensor.matmul(sp[:, 0:P], eye_sb[:],
                                             wedge_sb[:], start=True, stop=False)
                            nc.tensor.matmul(sp[:, 0:P], kTb,
                                             qv[:, qlo - q0:qlo - q0 + P],
                                             start=False, stop=True)
                            if n > P:
                                nc.tensor.matmul(sp[:, P:n], kTb,
                                                 qv[:, qlo - q0 + P:qlo - q0 + n],
                                                 start=True, stop=True)
                        else:
                            nc.tensor.matmul(sp[:, 0:n], kTb,
                                             qv[:, qlo - q0:qlo - q0 + n],
                                             start=True, stop=True)
                        ex = ex_pool.tile([P, QW], BF16, tag="ex", name="ex")
                        nc.scalar.activation(ex[:, 0:n], sp[:, 0:n],
                                             AF.Exp, scale=0.125)
                        exs.append((par, kc, qlo, n, ex))
                    # issue PV for the previous kc between this kc's scores
                    # and the next, so PE alternates scores/PV while the
                    # Activation engine streams exps one kc behind.
                    for args in pend_pv:
                        emit_pv(*args)
                    pend_pv = exs
                for args in pend_pv:
                    emit_pv(*args)
                mblk = slice(m * QW, (m + 1) * QW)
                with tc.high_priority():
                    for par in range(2):
                        with nc.allow_low_precision(reason="softmax sums"):
                            nc.vector.reciprocal(
                                stg_t[32 * par:32 * par + 1, mblk],
                                pvps[par][64:65, :])
                for par in range(2):
                    off = 64 * par
                    nc.scalar.copy(aoT[m][off:off + 64, :],
                                   pvps[par][0:64, :])
                # bp[j, q] = inv(head for row j): sel33 selects row 0 (j<64)
                # or row 32 (j>=64); rows 1..31 of stg_t are always zero
                bp = ring.tile([P, QW], F32, tag="ring", name="bp")
                nc.tensor.matmul(bp[:, :], sel_sb[:], stg_t[0:33, mblk],
                                 start=True, stop=True)
                nc.vector.tensor_mul(aoT[m][:, :], aoT[m][:, :], bp[:, :])

            def attn_post(qc, aoT, sums):
                q0 = qc * QW
                for t in range(QW // P):
                    s0 = q0 + t * P
                    ob = ob_pool.tile([P, HID], F32, tag="ob", name="ob")
                    for half in range(2):
                        c0, c1 = half * F, half * F + F
                        fin = ring.tile([P, QW], F32, tag="ring", name="fin")
                        for m in range(MC):
                            nc.tensor.matmul(fin[:, 0:F],
                                             aoT[m][:, s0 - q0:s0 - q0 + P],
                                             woT[m][:, c0:c1],
                                             start=(m == 0), stop=(m == MC - 1))
                        nc.scalar.copy(ob[:, c0:c1], fin[:, 0:F])
                    nc.sync.dma_start(out_d[s0:s0 + P, :], ob[:])

            def alloc_strip(qc):
                qTall = rope_pool.tile([P, MC * QW], BF16, tag="qTall", name="qTall")
                qTs = [qTall[:, m * QW:(m + 1) * QW] for m in range(MC)]
                aoT = [ao_pool.tile([P, QW], BF16, tag=f"aoTs{m}", name=f"aoTs{m}")
                       for m in range(MC)]
                return qTs, aoT, None, None

            # ---- emission: group 0, then zip(attention qc, proj group qc+1) ----
            strips = {0: alloc_strip(0)}
            pending = None
            pending_post = None
            for s in range(4):
                state = emit_proj(s)[:-1] + (strips[0][0],)
                if pending is not None:
                    emit_tail(pending)
                pending = state

            for qc in range(NQ):
                qTs, aoT, sums, stg = strips[qc]
                if qc + 1 < NQ:
                    strips[qc + 1] = alloc_strip(qc + 1)
                    nxt = list(range(4 * qc + 4, 4 * qc + 8))
                else:
                    nxt = []
                if not nxt and pending is not None:
                    emit_tail(pending)
                    pending = None
                for mp in range(MC + 1):
                    for s_i in nxt[mp:mp + 1]:
                        state = emit_proj(s_i)[:-1] + (strips[qc + 1][0],)
                        if pending is not None:
                            emit_tail(pending)
                        pending = state
                    if mp == 1 and pending_post is not None:
                        attn_post(*pending_post)
                        pending_post = None
                    if mp < MC:
                        attn_pair(qc, mp, qTs, aoT, sums, stg)
                pending_post = (qc, aoT, sums)
            attn_post(*pending_post)
    nc.compile()
    return nc


def _rope_tables():
    inv_freq = 1.0 / (ROPE_THETA ** (np.arange(0, D, 2, dtype=np.float32) / D))
    t = np.arange(S, dtype=np.float32)
    freqs = np.outer(t, inv_freq)                       # [S, 32]
    emb = np.concatenate([freqs, freqs], axis=-1)       # [S, 64]
    cos = np.cos(emb).astype(np.float32)
    sin = np.sin(emb).astype(np.float32)
    sin_signed = sin.copy()
    sin_signed[:, 0:32] *= -1.0                         # fold rotate_half sign
    # pack: tbl[p, s, d] = x[s*128 + p, d]  -> [128, 16*64]
    cosP = np.ascontiguousarray(
        cos.reshape(SC, P, D).transpose(1, 0, 2).reshape(P, SC * D))
    sinP = np.ascontiguousarray(
        sin_signed.reshape(SC, P, D).transpose(1, 0, 2).reshape(P, SC * D))
    return cosP, sinP


_STATE = {}


def _get_program():
    if "nc" not in _STATE:
        _STATE["nc"] = build_program()
    return _STATE["nc"]


def _bf16(x):
    import ml_dtypes
    return np.asarray(x, dtype=np.float32).astype(ml_dtypes.bfloat16)


def _make_in_maps(hidden_states, Wq, Wk, Wv, Wo):
    hs = np.asarray(hidden_states, dtype=np.float32)
    Wq = np.asarray(Wq, dtype=np.float32)
    Wk = np.asarray(Wk, dtype=np.float32)
    Wv = np.asarray(Wv, dtype=np.float32)
    Wo = np.asarray(Wo, dtype=np.float32)

    cosP, sinP = _rope_tables()
    eye = _bf16(np.eye(P, dtype=np.float32))
    wedge = _bf16(np.tril(np.full((P, P), MASKV, dtype=np.float32), k=-1))
    ones6 = _bf16(np.ones((P, NH), dtype=np.float32))
    sel33 = np.zeros((33, P), dtype=np.float32)
    sel33[0, 0:64] = 1.0
    sel33[32, 64:128] = 1.0

    in_maps = []
    for c in range(N_CORES):
        b, g = c // 2, c % 2
        cols = slice(g * F, (g + 1) * F)
        wT = np.concatenate([Wq[cols, :].T, Wk[cols, :].T, Wv[cols, :].T],
                            axis=1)                            # [768, 1152]
        in_maps.append({
            "xT": _bf16(np.ascontiguousarray(hs[b].T)),        # [768, S]
            "wT": _bf16(np.ascontiguousarray(wT)),
            "woT": _bf16(np.ascontiguousarray(Wo[:, cols].T)),  # [384, 768]
            "cosP": cosP,
            "sinP": sinP,
            "eye": eye,
            "wedge": wedge,
            "ones6": ones6,
            "sel33": sel33,

        })
    return in_maps


def run(hidden_states, Wq, Wk, Wv, Wo, trace=False, **trace_kw):
    nc = _get_program()
    in_maps = _make_in_maps(hidden_states, Wq, Wk, Wv, Wo)
    res = run_bass_kernel_spmd(nc, in_maps, core_ids=list(range(N_CORES)),
                               trace=trace, **trace_kw)
    B = 4
    out = np.empty((B, S, HID), dtype=np.float32)
    for b in range(B):
        out[b] = res.results[2 * b]["out"] + res.results[2 * b + 1]["out"]
    return out, res


def kernel(hidden_states, Wq, Wk, Wv, Wo):
    out, _ = run(hidden_states, Wq, Wk, Wv, Wo,
                 trace=bool(int(os.environ.get("KERNEL_TRACE", "0"))))
    return out
